# revision 98
# baseline (speedup 1.0000x reference)
"""Multi-head graph attention kernel for Trainium2 (8 NeuronCores).

Problem: B=8, N=1024, F_IN=F_OUT=128, H=8, D_K=16, sparse 0/1 adjacency mask.
Sharding: data-parallel over B — core b processes batch element b.

Math (identical to reference up to fp rounding):
    Q = X@Wq.T + bq ; K = X@Wk.T + bk ; V = X@Wv.T + bv
    S = Q_h @ K_h.T / 4 ;  P = exp(S) * A   (masked_fill(-1e9)+softmax ==
    zeroing masked entries of exp(S); softmax is shift-invariant and
    |S/4| < ~3 here so no max-subtract is needed)
    out = (P @ V_h) / rowsum(P) @ Wo.T + bo
    bv is folded into the output bias: out = (P@V0)/rs @ Wo.T + (bo + Wo@bv).

Device layout (everything transposed on host so the device never transposes):
    xt  [128c, 1024n]  = X.T                       (bf16)
    at  [1024k, 1024q] = A.T                       (bf16, 0/1 values)
    Heads are processed in two groups g in {0,1} of 4 heads; head slot a in
    {0..3} lives at partition base 32a with rows 16..31 zero-padded.
    Scores are computed transposed: ST[k, q] so the P@V matmul can stream
    P.T directly, with a leading ones column on V giving each head's
    P-rowsum at 32-aligned partitions for free. The mask is applied as a
    bf16 multiply on DVE after the exp (no PSUM mask-init matmuls), and the
    rowsum reciprocal is broadcast to all head partitions with a selector
    matmul (no DRAM round trips). Redundant same-engine semaphore waits are
    stripped post-hoc (in-order engines), and emission is software-pipelined
    so the ACT engine (the exp bottleneck: 8 heads x 1024^2 columns through
    a single 128-lane 1.2 GHz spline unit) stays busy.
"""

import sys

sys.path.insert(0, "/opt/trn_rl_repo")

import os

import numpy as np
import ml_dtypes

BF16 = ml_dtypes.bfloat16

# ablation switches for bottleneck isolation (defaults = full computation)
K_MASK = int(os.environ.get("K_MASK", "1"))
K_EXP = int(os.environ.get("K_EXP", "1"))
K_NORM = int(os.environ.get("K_NORM", "1"))
K_PV = int(os.environ.get("K_PV", "1"))
K_DROPSW = int(os.environ.get("K_DROPSW", "1"))

B, N, C, F, H, D = 8, 1024, 128, 128, 8, 16
NB = N // 128  # 8 k-blocks

_CACHED = None


def _split_multi_waits(nc):
    """This toolchain's walrus accepts at most ONE sync wait per instruction.
    First coalesce waits on the same counting semaphore (keep the max
    threshold — they subsume each other), then split any remaining extras
    onto preceding same-engine NOPs."""
    import concourse.mybir as mybir

    COMPUTE = ("PE", "Activation", "DVE", "Pool")

    for f in nc.m.functions:
        for blk in f.blocks:
            new = []
            for inst in blk.instructions:
                si = inst.sync_info
                eng = str(inst.engine).split(".")[-1]
                if (K_DROPSW and si is not None and si.on_wait
                        and eng in COMPUTE):
                    # drop waits on this engine's own completion semaphore:
                    # compute engines execute and complete in order, so a
                    # same-engine dependency is enforced by program order
                    kept = [w for w in si.on_wait
                            if not (w.sync_type == "semaphore"
                                    and w.wait_mode == "sem-ge-imm"
                                    and w.wait_reg is None
                                    and (w.ant_name or "").startswith(eng + "_"))]
                    if len(kept) != len(si.on_wait):
                        inst.sync_info = mybir.SyncInfo(
                            on_wait=kept, on_update=list(si.on_update or []))
                        si = inst.sync_info
                if si is not None and si.on_wait is not None and len(si.on_wait) > 1:
                    merged = {}
                    rest = []
                    for w in si.on_wait:
                        if (w.sync_type == "semaphore"
                                and w.wait_mode == "sem-ge-imm"
                                and w.wait_reg is None):
                            key = (w.id, w.ant_name)
                            if (key not in merged
                                    or merged[key].wait_value < w.wait_value):
                                merged[key] = w
                        else:
                            rest.append(w)
                    coalesced = list(merged.values()) + rest
                    inst.sync_info = mybir.SyncInfo(
                        on_wait=coalesced, on_update=list(si.on_update or []))
                    si = inst.sync_info
                if si is not None and si.on_wait is not None and len(si.on_wait) > 1:
                    waits = list(si.on_wait)
                    for w in waits[:-1]:
                        nop = mybir.InstNoOp(
                            name=nc.get_next_instruction_name(), ins=[], outs=[])
                        nop.engine = inst.engine
                        nop.sync_info = mybir.SyncInfo(on_wait=[w], on_update=[])
                        new.append(nop)
                    inst.sync_info = mybir.SyncInfo(
                        on_wait=[waits[-1]], on_update=list(si.on_update or []))
                new.append(inst)
            del blk.instructions[:]
            for i in new:
                blk.instructions.append(i)


def _build_nc(repeat=1, unroll=1):
    import concourse.bass as bass
    import concourse.tile as tile
    from concourse import mybir

    f32 = mybir.dt.float32
    f32r = mybir.dt.float32r
    bf16 = mybir.dt.bfloat16
    AF = mybir.ActivationFunctionType

    nc = bass.Bass()

    xt_d = nc.declare_dram_parameter("xt", [C, N], bf16, isOutput=False)
    at_d = nc.declare_dram_parameter("at", [N, N], bf16, isOutput=False)
    wqt_d = nc.declare_dram_parameter("wqt", [2, C, 128], bf16, isOutput=False)
    wkt_d = nc.declare_dram_parameter("wkt", [2, C, 128], bf16, isOutput=False)
    wvt_d = nc.declare_dram_parameter("wvt", [C, F], bf16, isOutput=False)
    wot_d = nc.declare_dram_parameter("wot", [2, 128, F], bf16, isOutput=False)
    bq_d = nc.declare_dram_parameter("bq2", [2, 128, 1], f32, isOutput=False)
    bk_d = nc.declare_dram_parameter("bk2", [2, 128, 1], f32, isOutput=False)
    bfin_d = nc.declare_dram_parameter("bfin", [F, 1], f32, isOutput=False)
    sel128_d = nc.declare_dram_parameter("sel128", [128, 128], f32,
                                         isOutput=False)
    yt_d = nc.declare_dram_parameter("yt", [F, N], f32, isOutput=True)
    dbg = int(os.environ.get("K_DBG", "0"))
    if dbg:
        dqt_d = nc.declare_dram_parameter("dqt", [2, 128, N], bf16, isOutput=True)
        dkt_d = nc.declare_dram_parameter("dkt", [2, 128, N], bf16, isOutput=True)
        dva_d = nc.declare_dram_parameter("dva", [128, NB * H * 32], bf16,
                                          isOutput=True)
        dot_d = nc.declare_dram_parameter("dot", [2, 128, N], bf16, isOutput=True)

    with tile.TileContext(nc) as tc:
        with tc.tile_pool(name="consts", bufs=1) as cp:
            xt_sb = cp.tile([C, N], bf16, name="xt_sb")
            nc.sync.dma_start(out=xt_sb[:], in_=xt_d[:, :])

            at_sb = []
            for j in range(NB):
                t = cp.tile([128, N], bf16, name=f"at_sb{j}")
                nc.sync.dma_start(out=t[:], in_=at_d[j * 128 : (j + 1) * 128, :])
                at_sb.append(t)

            wq_sb, wk_sb, wo_sb, bq_sb, bk_sb = [], [], [], [], []
            for g in range(2):
                w = cp.tile([C, 128], bf16, name=f"wq_sb{g}")
                nc.sync.dma_start(out=w[:], in_=wqt_d[g, :, :])
                wq_sb.append(w)
                w = cp.tile([C, 128], bf16, name=f"wk_sb{g}")
                nc.sync.dma_start(out=w[:], in_=wkt_d[g, :, :])
                wk_sb.append(w)
                w = cp.tile([128, F], bf16, name=f"wo_sb{g}")
                nc.sync.dma_start(out=w[:], in_=wot_d[g, :, :])
                wo_sb.append(w)
                b = cp.tile([128, 1], f32, name=f"bq_sb{g}")
                nc.sync.dma_start(out=b[:], in_=bq_d[g, :, :])
                bq_sb.append(b)
                b = cp.tile([128, 1], f32, name=f"bk_sb{g}")
                nc.sync.dma_start(out=b[:], in_=bk_d[g, :, :])
                bk_sb.append(b)
            wv_sb = cp.tile([C, F], bf16, name="wv_sb")
            nc.sync.dma_start(out=wv_sb[:], in_=wvt_d[:, :])
            bfin_sb = cp.tile([F, 1], f32, name="bfin_sb")
            nc.sync.dma_start(out=bfin_sb[:], in_=bfin_d[:, :])

            # V augmented with a LEADING ones column per head: [k, j, h, 32].
            # The ones column at lhsT index 0 puts each head's P-rowsum at
            # out partition 32a (32-aligned, so engines can touch it); V dims
            # follow at 1..16. Columns 17..31 are ALSO ones so the PV matmul
            # fills the whole 32-row band with finite values (rowsum copies)
            # — the normalize reciprocal would turn stale-zero PSUM in
            # unwritten rows into inf, and the selector matmul's 0-weighted
            # contraction of inf yields NaN.
            vaug_sb = cp.tile([128, NB, H, 32], bf16, name="vaug_sb")
            nc.vector.memset(vaug_sb[:, :, :, 0:1], 1.0)
            nc.vector.memset(vaug_sb[:, :, :, D + 1 : 32], 1.0)

            # rowsum broadcast selector: sel128[c, p] = 1 iff c == 32*(p//32).
            # matmul(lhsT=sel128, rhs=recip(ov)) replicates each band's
            # rowsum-reciprocal row to the whole band and zeroes junk rows.
            sel128_sb = cp.tile([128, 128], f32, name="sel128_sb")
            nc.sync.dma_start(out=sel128_sb[:], in_=sel128_d[:, :])

            # Per-group normalized head outputs OT[hd, q] (bf16). Partition
            # rows 32a+16..32a+31 are garbage, harmless: the matching rows of
            # wot are zero.
            otn_sb = [cp.tile([128, N], bf16, name=f"otn_sb{g}") for g in range(2)]

            yt_sb = cp.tile([F, N], f32, name="yt_sb")

            # QT/KT per group, head a at partitions 32a..32a+15 (16..31 zero)
            qt_sb = [cp.tile([128, N], bf16, name=f"qt_sb{g}") for g in range(2)]
            kt_sb = [cp.tile([128, N], bf16, name=f"kt_sb{g}") for g in range(2)]

            def emit(chain):
                with (
                    tc.tile_pool(name="s_ps", bufs=2, space="PSUM") as sp,
                    tc.tile_pool(name="ov_ps", bufs=2, space="PSUM") as op_,
                    tc.tile_pool(name="rs_ps", bufs=1, space="PSUM") as rp,
                    tc.tile_pool(name="v_ps", bufs=1, space="PSUM") as vp,
                    tc.tile_pool(name="ptp", bufs=int(os.environ.get("K_PTB", "16"))) as ptp,
                    tc.tile_pool(name="smalls", bufs=int(os.environ.get("K_SMB", "2"))) as smp,
                ):
                    def proj_qk(g):
                        qps = sp.tile([128, N], f32, tag="s")
                        nc.tensor.matmul(qps[:, 0:512], lhsT=wq_sb[g][:],
                                         rhs=xt_sb[:, 0:512])
                        nc.tensor.matmul(qps[:, 512:N], lhsT=wq_sb[g][:],
                                         rhs=xt_sb[:, 512:N])
                        nc.vector.tensor_scalar_add(qt_sb[g][:], qps[:],
                                                    bq_sb[g][:])
                        kps = sp.tile([128, N], f32, tag="s")
                        nc.tensor.matmul(kps[:, 0:512], lhsT=wk_sb[g][:],
                                         rhs=xt_sb[:, 0:512])
                        nc.tensor.matmul(kps[:, 512:N], lhsT=wk_sb[g][:],
                                         rhs=xt_sb[:, 512:N])
                        nc.vector.tensor_scalar_add(kt_sb[g][:], kps[:],
                                                    bk_sb[g][:])

                    def proj_v():
                        # 4 n-blocks of V per [128,512] PSUM tile, one copy each
                        for half in range(2):
                            vps = vp.tile([128, 512], f32, tag="vps")
                            for s in range(4):
                                j = 4 * half + s
                                nc.tensor.matmul(
                                    vps[:, s * 128 : (s + 1) * 128],
                                    lhsT=xt_sb[:, j * 128 : (j + 1) * 128],
                                    rhs=wv_sb[:])
                            nc.vector.tensor_copy(
                                out=vaug_sb[:, 4 * half : 4 * half + 4, :,
                                            1 : D + 1],
                                in_=vps[:].rearrange("p (s h d) -> p s h d",
                                                     h=H, d=D))

                    def emit_scores(g, q0, j):
                        """4 heads' transposed score slabs for k-block j:
                        two [128,1024] PSUM tiles, each holding 2 heads."""
                        spss = []
                        for pair in range(2):
                            sps = sp.tile([128, 1024], f32, tag="s")
                            for ai in range(2):
                                a = pair * 2 + ai
                                nc.tensor.matmul(
                                    sps[:, ai * 512 : (ai + 1) * 512],
                                    lhsT=kt_sb[g][32 * a : 32 * a + 32,
                                                  j * 128 : (j + 1) * 128],
                                    rhs=qt_sb[g][32 * a : 32 * a + 32,
                                                 q0 : q0 + 512],
                                    start=True, stop=True,
                                    skip_group_check=True,
                                    tile_position=(32 * a, 0),
                                )
                            spss.append(sps)
                        return spss

                    blocks = ((0, 0), (0, 1), (1, 0), (1, 1))
                    st = {}

                    def prologue():
                        """Group-0 projections + the first score tile of the
                        next logical iteration."""
                        proj_qk(0)
                        st["ov"] = op_.tile([128, 512], f32, tag="ov",
                                            name="ov")
                        st["spss"] = emit_scores(0, 0, 0)

                    # software-pipelined across blocks AND (within the
                    # unrolled For_i body) across logical iterations: the
                    # NEXT block's ov tile + first score tile — or, at the
                    # interior unroll seam, the next iteration's prologue —
                    # are emitted before the current block's normalize, so
                    # the ACT engine only waits at the loop edge.
                    def one_iter(first, last):
                      if first:
                        prologue()
                      for bi, (g, qh) in enumerate(blocks):
                        if True:
                            q0 = qh * 512
                            cur_ov = st["ov"]
                            if bi == 0 and first:
                                # loop-edge iteration: V and group-1 QK
                                # projections hide behind group-0 attention;
                                # interior iterations get them hoisted into
                                # the previous iteration's tail instead
                                proj_v()
                                proj_qk(1)
                            for j in range(NB):
                                # per score pair: exp into a [128,1024] bf16
                                # tile, then ONE mask multiply (mask slab
                                # repeated 2x via a stride-0 AP, non-in-place)
                                spss = st["spss"]
                                pts = []
                                for pair in range(2):
                                    pt = ptp.tile([128, 1024], bf16, tag="pt")
                                    if K_EXP:
                                        nc.scalar.activation(out=pt[:],
                                                             in_=spss[pair][:],
                                                             func=AF.Exp,
                                                             scale=0.25)
                                    else:
                                        nc.vector.memset(pt[:], 0.5)
                                    if K_MASK:
                                        mrow = at_sb[j][:, q0 : q0 + 512]
                                        m2 = bass.AP(
                                            tensor=mrow.tensor,
                                            offset=mrow.offset,
                                            ap=[list(mrow.ap[0]), [0, 2],
                                                list(mrow.ap[1])])
                                        pm = ptp.tile([128, 1024], bf16,
                                                      tag="pm")
                                        nc.vector.tensor_mul(pm[:], pt[:], m2)
                                        pt = pm
                                    pts.append(pt)
                                if j + 1 < NB:
                                    st["spss"] = emit_scores(g, q0, j + 1)
                                elif bi + 1 < len(blocks):
                                    ng, nqh = blocks[bi + 1]
                                    st["ov"] = op_.tile([128, 512], f32,
                                                        tag="ov", name="ov")
                                    st["spss"] = emit_scores(ng, nqh * 512, 0)
                                elif not last:
                                    # interior unroll seam: the next logical
                                    # iteration's prologue runs while this
                                    # one's tail (normalize, Y, store) drains
                                    prologue()
                                if K_PV or j == 0:
                                    last_j = NB - 1 if K_PV else 0
                                    for a in range(4):
                                        nc.tensor.matmul(
                                            cur_ov[32 * a : 32 * a + 32, :],
                                            lhsT=vaug_sb[:, j, 4 * g + a, :],
                                            rhs=pts[a // 2][
                                                :, (a % 2) * 512 : (a % 2) * 512 + 512],
                                            start=(j == 0),
                                            stop=(j == last_j),
                                            skip_group_check=True,
                                            tile_position=(0, 32 * a),
                                        )
                            if bi == 3 and not last:
                                # hoist the NEXT iteration's V and group-1 QK
                                # projections after the last PV (they
                                # overwrite vaug/qt1/kt1, which this
                                # iteration's PVs and scores just finished
                                # reading)
                                proj_v()
                                proj_qk(1)
                            # normalize: each head's P-rowsum sits at aligned
                            # partition 32a. Reciprocal the WHOLE ov tile
                            # (free-dim cost only; junk rows excluded below),
                            # broadcast row 32a to its band via the selector
                            # matmul, stage through SBUF, multiply on DVE.
                            if K_NORM:
                                rcf = smp.tile([128, 512], f32, tag="rcf")
                                nc.vector.reciprocal(out=rcf[:], in_=cur_ov[:])
                                rsb = rp.tile([128, 512], f32, tag="rsb")
                                nc.tensor.matmul(
                                    rsb[:], lhsT=sel128_sb[:], rhs=rcf[:],
                                    start=True, stop=True,
                                    skip_group_check=True,
                                )
                                rsc = smp.tile([128, 512], f32, tag="rsc")
                                nc.vector.tensor_copy(out=rsc[:], in_=rsb[:])
                                nc.vector.tensor_mul(
                                    otn_sb[g][:, q0 : q0 + 512], cur_ov[:],
                                    rsc[:])
                            else:
                                nc.vector.tensor_copy(
                                    out=otn_sb[g][:, q0 : q0 + 512],
                                    in_=cur_ov[:])

                        # ------------- output projection (per q-half) ------
                        # after the second group's normalize for this q-half;
                        # the qh=0 projection hides behind (g=1,qh=1) attention
                        if g == 1:
                            yps = vp.tile([128, 512], f32, tag="vps")
                            nc.tensor.matmul(yps[:], lhsT=wo_sb[0][:],
                                             rhs=otn_sb[0][:, q0 : q0 + 512],
                                             start=True, stop=False,
                                             skip_group_check=True)
                            nc.tensor.matmul(yps[:], lhsT=wo_sb[1][:],
                                             rhs=otn_sb[1][:, q0 : q0 + 512],
                                             start=False, stop=True,
                                             skip_group_check=True)
                            nc.vector.tensor_scalar_add(
                                yt_sb[:, q0 : q0 + 512], yps[:], bfin_sb[:])
                            nc.sync.dma_start(out=yt_d[:, q0 : q0 + 512],
                                              in_=yt_sb[:, q0 : q0 + 512])

                    for _f, _l in chain:
                        one_iter(_f, _l)

            if repeat > 1:
                # several logical iterations per For_i body: each interior
                # seam gets the next iteration's prologue hoisted, so the
                # ACT bubble only occurs at the (divided-count) loop edge
                UNROLL = 4
                with tc.For_i(0, repeat // UNROLL, 1):
                    emit([(i == 0, i == UNROLL - 1) for i in range(UNROLL)])
            else:
                emit([(i == 0, i == unroll - 1) for i in range(unroll)])
                if dbg:
                    for g in range(2):
                        nc.sync.dma_start(out=dqt_d[g, :, :], in_=qt_sb[g][:])
                        nc.sync.dma_start(out=dkt_d[g, :, :], in_=kt_sb[g][:])
                        nc.sync.dma_start(out=dot_d[g, :, :], in_=otn_sb[g][:])
                    nc.sync.dma_start(
                        out=dva_d[:, :],
                        in_=vaug_sb[:].rearrange("p a b c -> p (a b c)"))

    _split_multi_waits(nc)
    return nc


def _prep_host(inputs):
    """Host-side layout prep. Returns per-core input maps."""
    X = np.asarray(inputs["X"], dtype=np.float32)
    A = np.asarray(inputs["A"], dtype=np.float32)
    Wq = np.asarray(inputs["Wq"], dtype=np.float32)
    bq = np.asarray(inputs["bq"], dtype=np.float32)
    Wk = np.asarray(inputs["Wk"], dtype=np.float32)
    bk = np.asarray(inputs["bk"], dtype=np.float32)
    Wv = np.asarray(inputs["Wv"], dtype=np.float32)
    bv = np.asarray(inputs["bv"], dtype=np.float32)
    Wo = np.asarray(inputs["Wo"], dtype=np.float32)
    bo = np.asarray(inputs["bo"], dtype=np.float32)

    # grouped/padded QK weights: wqt[g, c, 32a+d] = Wq[(4g+a)*16+d, c], d<16
    def qk_prep(W, b):
        W4 = W.reshape(2, 4, D, C)  # [g, a, d, c]
        wt = np.zeros((2, C, 4, 32), dtype=np.float32)
        wt[:, :, :, :D] = W4.transpose(0, 3, 1, 2)
        b4 = b.reshape(2, 4, D)
        bt = np.zeros((2, 4, 32), dtype=np.float32)
        bt[:, :, :D] = b4
        return (wt.reshape(2, C, 128).astype(BF16),
                bt.reshape(2, 128, 1).astype(np.float32))

    wqt, bq2 = qk_prep(Wq, bq)
    wkt, bk2 = qk_prep(Wk, bk)
    wvt = Wv.T.copy().astype(BF16)  # [c, f]
    # wot[g, 32a+1+d, f] = Wo[f, (4g+a)*16+d], d<16 (row 32a is the rowsum
    # slot, rows 32a+17..31 are padding; both zero)
    Wo4 = Wo.reshape(F, 2, 4, D)  # [f, g, a, d]
    wot = np.zeros((2, 4, 32, F), dtype=np.float32)
    wot[:, :, 1 : D + 1, :] = Wo4.transpose(1, 2, 3, 0)
    wot = wot.reshape(2, 128, F).astype(BF16)
    bfin = (bo + Wo @ bv).reshape(F, 1).astype(np.float32)

    XT = X.transpose(0, 2, 1).astype(BF16)  # [b, c, n]
    # transposed 0/1 mask
    AT = (A.transpose(0, 2, 1) > 0).astype(BF16)

    sel128 = np.zeros((128, 128), dtype=np.float32)
    for p in range(128):
        sel128[32 * (p // 32), p] = 1.0

    in_maps = []
    for b in range(B):
        in_maps.append({
            "xt": np.ascontiguousarray(XT[b]),
            "at": np.ascontiguousarray(AT[b]),
            "wqt": wqt, "wkt": wkt, "wvt": wvt, "wot": wot,
            "bq2": bq2, "bk2": bk2, "bfin": bfin, "sel128": sel128,
        })
    return in_maps


def run(inputs, trace=False):
    """Returns (output [B,N,F] float32, BassKernelResults)."""
    global _CACHED
    from concourse import bass_utils

    if _CACHED is None:
        _CACHED = _build_nc()
    nc = _CACHED
    in_maps = _prep_host(inputs)
    res = bass_utils.run_bass_kernel_spmd(
        nc, in_maps, core_ids=list(range(B)), trace=trace)
    out = np.stack([np.asarray(r["yt"], dtype=np.float32).T for r in res.results])
    return out, res


def kernel(**inputs):
    out, _ = run(inputs, trace=False)
    return out


def bench_loop(inputs, R=16385, reps=5):
    """Device-side For_i repeat. Per-kernel time is the slope between two
    LARGE repeat counts (R/4 and R) so the per-dispatch overhead (axon RPC,
    transfers — hundreds of ms with tens-of-ms variance) cancels without
    injecting its noise into the estimate."""
    import time
    from concourse import bass_utils

    in_maps = _prep_host(inputs)
    R1 = (R - 1) // 4 + 1

    def timed(nc, reps):
        ts = []
        for _ in range(reps):
            t0 = time.perf_counter()
            bass_utils.run_bass_kernel_spmd(nc, in_maps, core_ids=list(range(B)))
            ts.append(time.perf_counter() - t0)
        return ts

    nc1 = _build_nc(R1)
    ncR = _build_nc(R)
    timed(nc1, 2)  # warm both compiles
    timed(ncR, 2)
    t1s, tRs = [], []
    for _ in range(reps):
        t1s.extend(timed(nc1, 1))
        tRs.extend(timed(ncR, 1))
    t1, tR = min(t1s), min(tRs)
    per = (tR - t1) / (R - R1)
    return per, {"t1s": t1s, "tRs": tRs}


def bench(inputs, iters=20):
    """Time repeated on-device executions (inputs resident, outputs donated
    from device-side zeros). Returns (best_s, all_times)."""
    global _CACHED
    import time
    import jax
    import jax.numpy as jnp
    import numpy as np_
    from jax.sharding import Mesh, PartitionSpec
    from jax.experimental.shard_map import shard_map
    from concourse import bass2jax, mybir

    if _CACHED is None:
        _CACHED = _build_nc()
    nc = _CACHED
    in_maps = _prep_host(inputs)
    n_cores = len(in_maps)

    bass2jax.install_neuronx_cc_hook()
    partition_name = nc.partition_id_tensor.name if nc.partition_id_tensor else None
    in_names, out_names, out_avals, zero_shapes = [], [], [], []
    for alloc in nc.m.functions[0].allocations:
        if not isinstance(alloc, mybir.MemoryLocationSet):
            continue
        name = alloc.memorylocations[0].name
        if alloc.kind == "ExternalInput":
            if name != partition_name:
                in_names.append(name)
        elif alloc.kind == "ExternalOutput":
            out_names.append(name)
            shape = tuple(alloc.tensor_shape)
            dtype = mybir.dt.np(alloc.dtype)
            out_avals.append(jax.core.ShapedArray(shape, dtype))
            zero_shapes.append((shape, dtype))
    n_params = len(in_names)
    all_in_names = list(in_names) + list(out_names)
    if partition_name is not None:
        all_in_names.append(partition_name)
    donate = tuple(range(n_params, n_params + len(out_names)))

    def _body(*args):
        operands = list(args)
        if partition_name is not None:
            operands.append(bass2jax.partition_id_tensor())
        outs = bass2jax._bass_exec_p.bind(
            *operands,
            out_avals=tuple(out_avals),
            in_names=tuple(all_in_names),
            out_names=tuple(out_names),
            lowering_input_output_aliases=(),
            sim_require_finite=True,
            sim_require_nnan=True,
            nc=nc,
        )
        return tuple(outs)

    devices = jax.devices()[:n_cores]
    mesh = Mesh(np_.asarray(devices), ("core",))
    in_specs = (PartitionSpec("core"),) * (n_params + len(out_names))
    out_specs = (PartitionSpec("core"),) * len(out_names)
    fn = jax.jit(
        shard_map(_body, mesh=mesh, in_specs=in_specs, out_specs=out_specs,
                  check_rep=False),
        donate_argnums=donate, keep_unused=True)

    concat_in = [
        jax.device_put(
            np_.concatenate([np_.asarray(in_maps[c][nm]) for c in range(n_cores)],
                            axis=0))
        for nm in in_names
    ]

    def make_zeros():
        return [jnp.zeros((n_cores * s[0],) + tuple(s[1:]), d)
                for (s, d) in zero_shapes]

    def _chainN(n):
        def _bodyN(*args):
            ins = list(args[:n_params])
            outs = list(args[n_params:])
            for _ in range(n):
                outs = list(_body(*ins, *outs))
            return tuple(outs)
        return jax.jit(
            shard_map(_bodyN, mesh=mesh, in_specs=in_specs, out_specs=out_specs,
                      check_rep=False),
            donate_argnums=donate, keep_unused=True)

    def timed(f, reps):
        # warmup/compile
        jax.block_until_ready(f(*concat_in, *make_zeros()))
        ts = []
        for _ in range(reps):
            z = make_zeros()
            jax.block_until_ready(z)
            t0 = time.perf_counter()
            jax.block_until_ready(f(*concat_in, *z))
            ts.append(time.perf_counter() - t0)
        return min(ts)

    n_hi = iters
    t1 = timed(_chainN(1), 8)
    thi = timed(_chainN(n_hi), 5)
    per_exec = (thi - t1) / (n_hi - 1)
    return per_exec, {"t1": t1, f"t{n_hi}": thi}
